# revision 23
# baseline (speedup 1.0000x reference)
"""Trainium2 Bass kernel for nn_CvxDifflayer (batched PDHG LP solver).

Math per batch row (300 iterations):
    u = (x, z);  x on V=1012 edges, z on N=144 nodes
    x' = clip(x - tau*(y @ Kx), 0, 1)          Kx = [A; A_pos]  (288 x 1012)
    z' = clip(z - tau*c + tau*y2, 0, 1)        (K's z-block is [-0; -I])
    xbar = 2x' - x,  zbar = 2z' - z
    y' = relu(y + sigma*(xbar @ Kx^T - h) - sigma*(0|zbar))
    output z after 300 iters.

Device mapping (per core, 64-batch shard split into 2x32 interleaved
halves to hide serial latency; all matmul inputs bf16, fp32 state):
  - Edges permuted by grid-row band (3 bands of 4 grid rows); y rows
    interleaved (y1[n], y2[n]) by node. Both matmuls then touch only
    narrow contiguous spans: MM1 (y@Kx banded) is 8 matmuls/1148 rows;
    MM2 (xbar@Kx^T) is 8 span matmuls/590 rows + a 288-row h-row pass
    that doubles as the PSUM zeroing pass.
  - x-update on DVE (+GPSIMD for half the PSUM read), z path on
    GPSIMD/DVE, y-update on DVE with relu->bf16 on ACT.
  - feat-major operands produced by PE transposes (bf16, 1c/row).
"""

import sys

for _p in ("/opt/trn_rl_repo", "/opt/pypackages"):
    if _p not in sys.path:
        sys.path.insert(0, _p)

import numpy as np
import ml_dtypes

BF16 = ml_dtypes.bfloat16
KG = 12
N = 144
V = 1012
YR = 288
B = 512
BS = 64          # batch per core
HB = 32          # batch per half
N_CORES = 8
ITERS = 300
ROWS_PER_BAND = 4


def _edges():
    offs = [(p, q) for p in (-1, 0, 1) for q in (-1, 0, 1) if (p, q) != (0, 0)]
    es = []
    for i in range(KG):
        for j in range(KG):
            for p, q in offs:
                ii, jj = i + p, j + q
                if 0 <= ii < KG and 0 <= jj < KG:
                    es.append((i * KG + j, ii * KG + jj))
    return es


def _tables():
    es = _edges()

    def nb(node):
        return (node // KG) // ROWS_PER_BAND

    keys = [(min(nb(s), nb(t)), max(nb(s), nb(t)), s) for (s, t) in es]
    order = sorted(range(V), key=lambda e: keys[e])
    perm_e = np.array(order, dtype=np.int64)
    # segments of equal (b1,b2)
    segs = []
    st, cur = 0, keys[order[0]][:2]
    for j in range(1, V + 1):
        if j == V or keys[order[j]][:2] != cur:
            segs.append((cur[0], cur[1], st, j))
            if j < V:
                cur, st = keys[order[j]][:2], j
    # split any segment at col 512 (PSUM bank boundary)
    ss = []
    for (b1, b2, s0, s1) in segs:
        if s0 < 512 < s1:
            ss += [(b1, b2, s0, 512), (b1, b2, 512, s1)]
        else:
            ss.append((b1, b2, s0, s1))
    # mm1 list: (band, c0, c1, first_writer), A-bank cols first
    mm1 = []
    for bank in (0, 1):
        for b in range(3):
            for (b1, b2, s0, s1) in ss:
                if b in (b1, b2) and ((s1 <= 512) == (bank == 0)):
                    mm1.append((b, s0, s1, b == b1))
    # order mm1 so all bank-A writers come before bank-B (they already do),
    # and stop flags on last writer per bank
    y_perm = np.empty(YR, dtype=np.int64)
    y_perm[0::2] = np.arange(N)
    y_perm[1::2] = N + np.arange(N)
    chunks = []
    for c0 in range(0, V, 128):
        c1 = min(c0 + 128, V)
        nodes = set()
        for j in range(c0, c1):
            s, t = es[perm_e[j]]
            nodes.add(s)
            nodes.add(t)
        chunks.append((c0, c1, 2 * min(nodes), 2 * max(nodes) + 2))
    return perm_e, y_perm, mm1, chunks


PERM_E, Y_PERM, MM1_TAB, MM2_CHUNKS = _tables()
BAND_COL0 = [0, 292, 652]        # first x-col touched by band b
KA1_OFF = [0, 360, 788]          # col offset of band b block inside KA1
KA1_W = 1148
KS2_OFF = np.cumsum([0] + [hi - lo for (_, _, lo, hi) in MM2_CHUNKS]).tolist()
KS2_W = KS2_OFF[-1]
CB_KA1 = 0
CB_KS2 = CB_KA1 + KA1_W
CB_SHP = CB_KS2 + KS2_W
CB_W = CB_SHP + YR


def _host_consts(A, A_pos, b):
    Kx = np.concatenate([A, A_pos], axis=0).astype(np.float32)
    h = np.concatenate([b.astype(np.float32), np.zeros(N, np.float32)])
    # ||K||_1 / ||K||_inf of the full K (incl. z block) as in reference
    Kfull = np.concatenate(
        [np.concatenate([A, np.zeros((N, N), np.float32)], 1),
         np.concatenate([A_pos, -np.eye(N, dtype=np.float32)], 1)], 0)
    Kn = np.sqrt(np.abs(Kfull).sum(0).max() * np.abs(Kfull).sum(1).max())
    tau = np.float32(np.float32(0.9) / Kn)
    Kxp = Kx[np.ix_(Y_PERM, PERM_E)]
    hp = h[Y_PERM]
    cb = np.zeros((128, CB_W), np.float32)
    for bd in range(3):
        r0 = 96 * bd
        c0 = BAND_COL0[bd]
        w = (360, 428, 360)[bd]
        cb[0:96, CB_KA1 + KA1_OFF[bd]:CB_KA1 + KA1_OFF[bd] + w] = \
            tau * Kxp[r0:r0 + 96, c0:c0 + w]
    for ci, (c0, c1, ylo, yhi) in enumerate(MM2_CHUNKS):
        cb[0:c1 - c0, CB_KS2 + KS2_OFF[ci]:CB_KS2 + KS2_OFF[ci + 1]] = \
            tau * Kxp[ylo:yhi, c0:c1].T
    cb[0:1, CB_SHP:CB_SHP + YR] = -tau * hp[None, :]
    return cb.astype(BF16), tau


def _build_bass():
    from concourse import bass, mybir
    from concourse.tile import TileContext
    from concourse.tile_rust import add_dep_helper
    from concourse.mybir import AluOpType as op

    f32 = mybir.dt.float32
    bf16 = mybir.dt.bfloat16
    relu = mybir.ActivationFunctionType.Relu

    nc = bass.Bass()
    d_cb = nc.dram_tensor("cb", (128, CB_W), bf16, kind="ExternalInput")
    d_cf = nc.dram_tensor("cf", (32, 2 * N), f32, kind="ExternalInput")
    d_z = nc.dram_tensor("z", (64, N), f32, kind="ExternalOutput")

    with TileContext(nc) as tc:
        with (
            tc.tile_pool(name="state", bufs=1) as sp,
            tc.tile_pool(name="psGA_A", bufs=1, space="PSUM") as pGAa,
            tc.tile_pool(name="psGB_A", bufs=1, space="PSUM") as pGBa,
            tc.tile_pool(name="psGA_B", bufs=1, space="PSUM") as pGAb,
            tc.tile_pool(name="psGB_B", bufs=1, space="PSUM") as pGBb,
            tc.tile_pool(name="psTP_A", bufs=1, space="PSUM") as pTPa,
            tc.tile_pool(name="psTP_B", bufs=1, space="PSUM") as pTPb,
            tc.tile_pool(name="psPY_A", bufs=1, space="PSUM") as pPYa,
            tc.tile_pool(name="psPY_B", bufs=1, space="PSUM") as pPYb,
        ):
            CB = sp.tile([128, CB_W], bf16)
            CF = sp.tile([32, 2 * N], f32)
            KA1 = CB[:, CB_KA1:CB_KS2]
            KS2 = CB[:, CB_KS2:CB_SHP]
            SHP = CB[:, CB_SHP:CB_W]
            IDENTB = sp.tile([32, 32], f32)
            ONESB = sp.tile([1, 32], bf16)
            SCRD = sp.tile([32, 32], f32)
            SCRA = sp.tile([32, 16], f32)
            SCRP = sp.tile([32, 8], f32)
            NEGS = sp.tile([32, N], f32)

            halves = []
            for hn in ("A", "B"):
                H = {}
                H["X32"] = [sp.tile([HB, V], f32, tag=f"x32{hn}{i}")
                            for i in range(2)]
                H["XBAR"] = sp.tile([HB, V], bf16, tag=f"xbar{hn}")
                H["TMP"] = sp.tile([HB, V], f32, tag=f"tmp{hn}")
                H["Z32"] = [sp.tile([HB, N], f32, tag=f"z32{hn}{i}")
                            for i in range(2)]
                H["ZBAR"] = sp.tile([HB, N], f32, tag=f"zbar{hn}")
                H["TZ"] = sp.tile([HB, N], f32, tag=f"tz{hn}")
                H["PRESZ"] = sp.tile([HB, N], f32, tag=f"presz{hn}")
                H["Y32"] = [sp.tile([HB, YR], f32, tag=f"y32{hn}{i}")
                            for i in range(2)]
                H["T3"] = sp.tile([HB, YR], f32, tag=f"t3{hn}")
                H["YBF"] = sp.tile([HB, YR], bf16, tag=f"ybf{hn}")
                H["YFM"] = sp.tile([96, 96], bf16, tag=f"yfm{hn}")
                H["UFM"] = [sp.tile([128, 128], bf16, tag=f"ufm{hn}{i}")
                            for i in range(2)]
                halves.append(H)
            HA, HB_ = halves
            HA["TC"] = CF[:, 0:N]
            HB_["TC"] = CF[:, N:2 * N]

            # PSUM tiles
            HA["GA"] = pGAa.tile([HB, 512], f32, name="gaA")
            HA["GB"] = pGBa.tile([HB, 500], f32, name="gbA")
            HB_["GA"] = pGAb.tile([HB, 512], f32, name="gaB")
            HB_["GB"] = pGBb.tile([HB, 500], f32, name="gbB")
            HA["TPX"] = [pTPa.tile([128, 128], f32, tag=f"tpxA{i}")
                         for i in range(2)]
            HB_["TPX"] = [pTPb.tile([128, 128], f32, tag=f"tpxB{i}")
                          for i in range(2)]
            HA["P"] = pPYa.tile([HB, YR], f32, name="pA")
            HA["TY"] = pPYa.tile([96, 96], f32, tag="tyA")
            HB_["P"] = pPYb.tile([HB, YR], f32, name="pB")
            HB_["TY"] = pPYb.tile([96, 96], f32, tag="tyB")

            dma1 = nc.sync.dma_start(CB[:, :], d_cb[:, :])
            dma2 = nc.sync.dma_start(CF[:, :], d_cf[:, :])

            prev = {}

            def chain(eng, inst, *sync_deps):
                for d in sync_deps:
                    add_dep_helper(inst.ins, d.ins, True, "warm")
                if eng in prev:
                    add_dep_helper(inst.ins, prev[eng].ins, False, "order")
                prev[eng] = inst
                return inst

            def pe(inst, *d):
                return chain("pe", inst, *d)

            def dve(inst, *d):
                return chain("dve", inst, *d)

            def act(inst, *d):
                return chain("act", inst, *d)

            def gps(inst, *d):
                return chain("gps", inst, *d)

            # ---- init + engine warmups (absorb one foreign sem each) ----
            gps(nc.gpsimd.memset(IDENTB[:, :], 0.0))
            pool_last = gps(nc.gpsimd.affine_select(
                out=IDENTB[:, :], in_=IDENTB[:, :],
                compare_op=mybir.AluOpType.not_equal, fill=1.0, base=0,
                pattern=[[-1, 32]], channel_multiplier=1))
            dve(nc.vector.tensor_copy(SCRD[0:32, 28:32], CF[0:32, 0:4]))
            for H in halves:
                dve(nc.vector.tensor_scalar_mul(H["PRESZ"][:, :], H["TC"],
                                                -1.0))
            dve(nc.vector.memset(ONESB[:, :], 1.0))
            dve(nc.vector.memset(NEGS[:, :], -float(TAU[0])))
            for H in halves:
                H["x_init"] = dve(nc.vector.memset(H["X32"][0][:, :], 0.0))
                dve(nc.vector.memset(H["Z32"][0][:, :], 0.0))
                dve(nc.vector.memset(H["Y32"][0][:, :], 0.0))
                dve(nc.vector.memset(H["XBAR"][:, :], 0.0))
            dve(nc.vector.memset(HA["YFM"][:, :], 0.0))
            dve_last = dve(nc.vector.memset(HB_["YFM"][:, :], 0.0))
            gps(nc.gpsimd.tensor_copy(SCRP[0:32, 4:8], CF[0:32, 4:8]))
            act(nc.scalar.copy(SCRA[0:32, 0:2], IDENTB[0:32, 0:2]))
            act(nc.scalar.copy(SCRA[0:32, 2:4], CF[0:32, 8:10]))
            act(nc.scalar.copy(SCRA[0:32, 4:6], HB_["YFM"][0:32, 0:2]))
            pe(nc.tensor.transpose(HA["TPX"][0:32, 0:32], IDENTB[:, :],
                                   IDENTB[:, :]))
            pe(nc.tensor.transpose(HA["TPX"][0:32, 0:32],
                                   HA["XBAR"][:, 0:32], IDENTB[:, :]))
            pe(nc.tensor.matmul(HA["GA"][0:32, 0:64], CB[0:96, 0:32],
                                CB[0:96, 64:128], start=True, stop=True))

            # per-segment writer counts: a segment's last writer closes
            # its psum group (stop is sim-only bookkeeping)
            from collections import Counter
            seg_writers = Counter((c0, c1) for (_, c0, c1, _) in MM1_TAB)

            def emit_P1(H, it):
                # MM1: G = tau*(y @ Kx), banded. Returns last matmul of
                # each PSUM bank so readers can absorb all groups via one
                # explicit dep (ISA allows one sem wait per instruction).
                seen = Counter()
                lastA = lastB = None
                for (bd, c0, c1, first) in MM1_TAB:
                    seen[(c0, c1)] += 1
                    last = seen[(c0, c1)] == seg_writers[(c0, c1)]
                    if c1 <= 512:
                        out = H["GA"][:, c0:c1]
                    else:
                        out = H["GB"][:, c0 - 512:c1 - 512]
                    off = KA1_OFF[bd] + (c0 - BAND_COL0[bd])
                    mi = pe(nc.tensor.matmul(
                        out, H["YFM"][0:96, 32 * bd:32 * bd + 32],
                        KA1[0:96, off:off + (c1 - c0)],
                        start=first, stop=last,
                        skip_group_check=True))
                    if c1 <= 512:
                        lastA = mi
                    else:
                        lastB = mi
                H["mm1_last"] = (lastA, lastB)

            def absorb(site, dep):
                # a zero-data-dep op whose ONLY sem wait is the explicit
                # dep: reads a never-written col, writes a col owned by
                # this site alone (same-site WAW is outside the engine
                # OOO window one period later)
                i = dve(nc.vector.tensor_copy(
                    SCRD[0:32, 2 * site:2 * site + 1],
                    SCRD[0:32, 2 * site + 1:2 * site + 2]))
                add_dep_helper(i.ins, dep.ins, True, "absorb")
                return i

            def emit_P2(H, it):
                p, q = it % 2, (it + 1) % 2
                X0, X1 = H["X32"][p], H["X32"][q]
                if it == 0:
                    # X32 memset is still within the DVE OOO window here;
                    # pre-wait it so the stt carries only the PE wait
                    absorb(0 if H is HA else 2, H["x_init"])
                # x path (multi psum-group deps merge: same PE sem)
                dve(nc.vector.scalar_tensor_tensor(
                    H["TMP"][:, 0:512], H["GA"][:, :], -1.0, X0[:, 0:512],
                    op.mult, op.add))
                dve(nc.vector.scalar_tensor_tensor(
                    H["TMP"][:, 512:V], H["GB"][:, :], -1.0, X0[:, 512:V],
                    op.mult, op.add))
                dve(nc.vector.tensor_scalar(
                    X1[:, :], H["TMP"][:, :], 0.0, 1.0, op.max, op.min))
                dve(nc.vector.scalar_tensor_tensor(
                    H["XBAR"][:, :], X1[:, :], 2.0, X0[:, :],
                    op.mult, op.subtract))
                # z path (all DVE; 144-wide ops are cheap at 2x)
                Z0, Z1 = H["Z32"][p], H["Z32"][q]
                dve(nc.vector.scalar_tensor_tensor(
                    H["TZ"][:, :], H["Y32"][p][:, 1::2], float(TAU[0]),
                    H["PRESZ"][:, :], op.mult, op.add))
                dve(nc.vector.tensor_scalar(
                    Z1[:, :], H["TZ"][:, :], 0.0, 1.0, op.max, op.min))
                dve(nc.vector.scalar_tensor_tensor(
                    H["ZBAR"][:, :], Z1[:, :], 2.0, Z0[:, :],
                    op.mult, op.subtract))
                dve(nc.vector.tensor_sub(H["PRESZ"][:, :], Z1[:, :],
                                         H["TC"]))

            def emit_P3(H, it):
                # transposes of XBAR -> TPX -> UFM (bf16), MM2 into P
                s = 8 if H is HA else 11
                for g in range(2):
                    tlast = None
                    for k in range(4):
                        c = 4 * g + k
                        c0 = 128 * c
                        cw = min(128, V - c0)
                        tlast = pe(nc.tensor.transpose(
                            H["TPX"][0:cw, 32 * c:32 * c + 32],
                            H["XBAR"][:, c0:c0 + cw], IDENTB[:, :]))
                    i = act(nc.scalar.copy(SCRA[0:32, s + g:s + g + 1],
                                           SCRA[0:32, 14:15]))
                    add_dep_helper(i.ins, tlast.ins, True, "absorb")
                    act(nc.scalar.copy(H["UFM"][g][:, :],
                                       H["TPX"][:, 128 * g:128 * g + 128]))
                pe(nc.tensor.matmul(
                    H["P"][:, :], ONESB[0:1, :], SHP[0:1, :],
                    start=True, stop=True, skip_group_check=True))
                for ci, (c0, c1, ylo, yhi) in enumerate(MM2_CHUNKS):
                    g, k = ci // 4, ci % 4
                    cw = c1 - c0
                    mi = pe(nc.tensor.matmul(
                        H["P"][:, ylo:yhi],
                        H["UFM"][g][0:cw, 32 * k:32 * k + 32],
                        KS2[0:cw, KS2_OFF[ci]:KS2_OFF[ci + 1]],
                        start=False, stop=True,
                        skip_group_check=True))
                H["mm2_last"] = mi

            def emit_P4(H, it):
                p, q = it % 2, (it + 1) % 2
                dve(nc.vector.scalar_tensor_tensor(
                    H["T3"][:, :], H["P"][:, :], 1.0, H["Y32"][p][:, :],
                    op.mult, op.add))
                dve(nc.vector.scalar_tensor_tensor(
                    H["T3"][:, 1::2], H["ZBAR"][:, :], -float(TAU[0]),
                    H["T3"][:, 1::2], op.mult, op.add))
                act(nc.scalar.activation(H["Y32"][q][:, :], H["T3"][:, :],
                                         relu))

            def emit_P5(H, it):
                q = (it + 1) % 2
                tlast = None
                for bd in range(3):
                    tlast = pe(nc.tensor.transpose(
                        H["TPX"][0:96, 256 + 32 * bd:256 + 32 * bd + 32],
                        H["Y32"][q][:, 96 * bd:96 * bd + 96], IDENTB[:, :]))
                s5 = 10 if H is HA else 13
                i = act(nc.scalar.copy(SCRA[0:32, s5:s5 + 1],
                                       SCRA[0:32, 15:16]))
                add_dep_helper(i.ins, tlast.ins, True, "absorb")
                act(nc.scalar.copy(H["YFM"][:, :], H["TPX"][0:96, 256:352]))

            for it in range(ITERS):
                last = it == ITERS - 1
                emit_P1(HA, it)
                if it > 0:
                    emit_P3(HB_, it - 1)
                emit_P2(HA, it)
                if it > 0:
                    emit_P4(HB_, it - 1)
                    emit_P5(HB_, it - 1)
                emit_P1(HB_, it)
                if not last:
                    emit_P3(HA, it)
                emit_P2(HB_, it)
                if not last:
                    emit_P4(HA, it)
                    emit_P5(HA, it)

            zf = ITERS % 2
            zdma1 = nc.sync.dma_start(d_z[0:32, :], HA["Z32"][zf][:, :])
            zdma2 = nc.sync.dma_start(d_z[32:64, :], HB_["Z32"][zf][:, :])
            for d in (dma1, dma2, prev["gps"], prev["act"], prev["pe"],
                      prev["dve"], zdma1, zdma2):
                nn = nc.sync.nop()
                add_dep_helper(nn.ins, d.ins, True, "tail fence")
    return nc


TAU = np.zeros(1, np.float32)
LAST_RESULT = None


def kernel(weights, A, A_pos, b, _trace=False):
    global TAU
    weights = np.asarray(weights, np.float32)
    A = np.asarray(A, np.float32)
    A_pos = np.asarray(A_pos, np.float32)
    b = np.asarray(b, np.float32)

    cb, tau = _host_consts(A, A_pos, b)
    TAU[0] = tau

    nc = _build_bass()

    in_maps = []
    for core in range(N_CORES):
        w_shard = weights[core * BS:(core + 1) * BS].reshape(BS, N)
        tc = (tau * w_shard).astype(np.float32)
        cf = np.concatenate([tc[0:HB], tc[HB:BS]], axis=1)
        in_maps.append({"cb": cb, "cf": cf})

    from concourse.bass_utils import run_bass_kernel_spmd
    res = run_bass_kernel_spmd(nc, in_maps, core_ids=list(range(N_CORES)),
                               trace=_trace)
    global LAST_RESULT
    LAST_RESULT = res
    outs = [np.asarray(res.results[c]["z"]) for c in range(N_CORES)]
    z = np.concatenate(outs, axis=0).reshape(B, KG, KG)
    return z.astype(np.float32)


if __name__ == "__main__":
    TAU[0] = 0.1
    _build_bass()
    print("bass build OK")


# revision 24
# speedup vs baseline: 1.2752x; 1.2752x over previous
"""Trainium2 Bass kernel for nn_CvxDifflayer (batched PDHG LP solver).

Math (per batch row b):
    u_{k+1} = clip(u_k - tau*(q + y_k @ K), 0, 1)
    ubar    = 2*u_{k+1} - u_k
    y_{k+1} = relu(y_k + sigma*(ubar @ K.T - h))
    out z   = u_300[:, V:]  reshaped (12, 12)

Device reformulation (exact, per 64-batch shard):
    G_k  = tau*q + tau*(y_k @ K)        MM1: lhsT = Y feat-major, rhs = tau*K
    u    = clip(pres_k - G'_k)          where pres_k = u_k - tau*q, G' = tau*(yK)
    pres = u - tau*q                    (GPSIMD, off critical path)
    P_k  = sigma*(u @ K.T) - sigma*h    MM2: lhsT = u feat-major (PE transposes),
                                        h folded via constant ones-row in lhsT
    y    = relu(y + 2*P_k - P_{k-1})    (P_{-1} = -sigma*h since u_0 = 0)

Layouts per core (batch shard Bs=64):
    U     [128, 578]  batch-major fold: row 64*hf+b, col j = feature 578*hf+j
    UFM   [128, 640]  feat-major: chunk c cols [128c:128c+128); within chunk,
                      col 64*h+b = batch b of fold-half h; rows = fold-col
                      128c+i; chunk 4 rows 0:66 (+ row 66 = ones for h-fold)
    YBM   [64, 288]   batch-major y
    YFM   [128, 192]  feat-major y: block ci cols [64ci:64ci+64) = batch,
                      rows = y-row 128ci+i
"""

import sys

for _p in ("/opt/trn_rl_repo", "/opt/pypackages"):
    if _p not in sys.path:
        sys.path.insert(0, _p)

import numpy as np

N_GRID = 12
N = 144          # nodes
V = 1012         # directed edges
F = V + N        # 1156 primal vars
YR = 2 * N       # 288 dual vars
B = 512
BS = 64          # batch per core
N_CORES = 8
ITERS = 300
FPAD = 1280      # MM1 free dim padded so all chunks >= 256 (fp32r rate)
FM_CHUNKS = 10   # ceil(1156/128)
LAST_CW = F - 9 * 128      # 4


def _build_constants(A, A_pos, b):
    K = np.zeros((YR, F), np.float32)
    K[:N, :V] = A
    K[N:, :V] = A_pos
    K[N:, V:] = -np.eye(N, dtype=np.float32)
    h = np.concatenate([b.astype(np.float32), np.zeros(N, np.float32)])
    Kn = np.float32(np.sqrt(np.abs(K).sum(0).max() * np.abs(K).sum(1).max()))
    tau = np.float32(0.9) / Kn
    return K, h, tau


def _host_tiles(K, h, tau):
    """Constant SBUF images shared by all cores."""
    sigma = tau
    tauK = (tau * K).astype(np.float32)          # (288, 1156)
    sigK = (sigma * K).astype(np.float32)

    # KA1: MM1 rhs, 3 contraction chunks side by side, free dim padded to
    # FPAD so every matmul free-chunk is >=256 (full fp32r rate)
    ka1 = np.zeros((128, 3 * FPAD), np.float32)
    for r in range(3):
        r0 = 128 * r
        rw = min(128, YR - r0)
        ka1[:rw, FPAD * r:FPAD * r + F] = tauK[r0:r0 + rw, :]

    # KS2: MM2 rhs, 10 feat chunks of [rows, 288] side by side
    ks2 = np.zeros((128, 10 * YR), np.float32)
    for c in range(FM_CHUNKS):
        cw = 128 if c < 9 else LAST_CW
        f0 = 128 * c
        ks2[:cw, YR * c:YR * c + YR] = sigK[:, f0:f0 + cw].T
        if c == 9:
            ks2[LAST_CW, YR * c:YR * c + YR] = -sigma * h  # ones-row fold
    return ka1, ks2


def _per_core_tiles(w_shard, tau):
    """tq for one 64-row batch shard; w_shard (64, 144)."""
    tq = np.zeros((64, F), np.float32)
    tq[:, V:] = tau * w_shard
    return tq


# constsr layout (f32r, read-only): ka1 | ks2
C_KA1 = 0
C_KS2 = C_KA1 + 3 * FPAD
CR_W = C_KS2 + 10 * YR
# constsf layout (f32, read-only): tq | yp0  (yp0 = +sigma*h)
C_TQ = 0
C_YP = C_TQ + F
CF_W = C_YP + YR

FA = 512          # feature split: A = 0:512, B = 512:1156
FB = F - FA       # 644


def _pack_consts(ka1, ks2, tq, yp0):
    cr = np.zeros((128, CR_W), np.float32)
    cr[:, C_KA1:C_KS2] = ka1
    cr[:, C_KS2:CR_W] = ks2
    cf = np.zeros((64, CF_W), np.float32)
    cf[:, C_TQ:C_YP] = tq
    cf[:, C_YP:CF_W] = yp0
    return cr, cf


def _build_bass():
    from concourse import bass, mybir
    from concourse.tile import TileContext
    from concourse.tile_rust import add_dep_helper
    from concourse.mybir import AluOpType as op

    f32 = mybir.dt.float32
    f32r = mybir.dt.float32r

    nc = bass.Bass()
    d_cr = nc.dram_tensor("constsr", (128, CR_W), f32r, kind="ExternalInput")
    d_cf = nc.dram_tensor("constsf", (64, CF_W), f32, kind="ExternalInput")
    d_z = nc.dram_tensor("z", (64, N), f32, kind="ExternalOutput")

    with TileContext(nc) as tc:
        with (
            tc.tile_pool(name="state", bufs=1) as sp,
            tc.tile_pool(name="psA", bufs=1, space="PSUM") as psA,
            tc.tile_pool(name="psB", bufs=1, space="PSUM") as psB,
            tc.tile_pool(name="psP", bufs=1, space="PSUM") as psP,
            tc.tile_pool(name="psT0", bufs=1, space="PSUM") as psT0,
            tc.tile_pool(name="psT1", bufs=1, space="PSUM") as psT1,
            tc.tile_pool(name="psTY", bufs=1, space="PSUM") as psTY,
        ):
            CONSTR = sp.tile([128, CR_W], f32r)
            CONSTF = sp.tile([64, CF_W], f32)
            KA1 = CONSTR[:, C_KA1:C_KS2]
            KS2 = CONSTR[:, C_KS2:CR_W]
            TQ_A = CONSTF[:, C_TQ:C_TQ + FA]
            TQ_B = CONSTF[:, C_TQ + FA:C_YP]
            U_A = sp.tile([64, FA], f32)
            U_B = sp.tile([64, FB], f32)
            TMP_A = sp.tile([64, FA], f32)
            TMP_B = sp.tile([64, FB], f32)
            PRES_A = sp.tile([64, FA], f32)
            PRES_B = sp.tile([64, FB], f32)
            YP = sp.tile([64, YR], f32)
            YBM = sp.tile([64, YR], f32)
            T3 = sp.tile([64, YR], f32)
            # per-engine scratch tiles (separate so absorber ops never
            # create cross-engine tile deps)
            SCRD = sp.tile([32, 8], f32)
            SCRA = sp.tile([32, 12], f32)
            SCRP = sp.tile([32, 8], f32)
            ONES32 = sp.tile([32, 64], f32)
            ZER128 = sp.tile([128, 192], f32)
            # feat-major u in two wide tiles; ones-row for the h-fold at
            # row LAST_CW of the chunk-9 column block of UFM1
            UFM0 = sp.tile([128, 256], f32r)   # chunks 0..3
            UFM45 = sp.tile([128, 128], f32r)  # chunks 4,5
            UFM67 = sp.tile([128, 128], f32r)  # chunks 6,7
            UFM8 = sp.tile([128, 64], f32r)    # chunk 8
            UFM9 = sp.tile([32, 64], f32r)     # chunk 9 + ones row
            YFMb = []
            for c in range(3):
                yfm_c = sp.tile([128, 64], f32r, tag=f"yfm{c}")
                YFMb.append(yfm_c)
            IDENT = sp.tile([128, 128], f32)

            dma1 = nc.sync.dma_start(CONSTR[:, :], d_cr[:, :])
            dma2 = nc.sync.dma_start(CONSTF[:, :], d_cf[:, :])

            pool_insts = [
                nc.gpsimd.memset(IDENT[:, :], 0.0),
                nc.gpsimd.affine_select(
                    out=IDENT[:, :], in_=IDENT[:, :],
                    compare_op=mybir.AluOpType.not_equal, fill=1.0, base=0,
                    pattern=[[-1, 128]], channel_multiplier=1),
            ]
            dve_insts = [
                nc.vector.memset(U_A[:, :], 0.0),
                nc.vector.memset(U_B[:, :], 0.0),
                nc.vector.memset(ONES32[:, :], 1.0),
                nc.vector.memset(ZER128[:, :], 0.0),
                nc.vector.memset(YBM[:, :], 0.0),
            ]

            G_A = psA.tile([64, FA], f32)
            G_B = psB.tile([64, 768], f32)
            P = psP.tile([64, YR], f32)
            TPX = psT0.tile([128, 256], f32)   # chunks 0..3, then 8..9
            TP45 = psT1.tile([128, 128], f32)
            TP67 = psT1.tile([128, 128], f32, tag="tp67")
            TY = psTY.tile([128, 192], f32)

            # This target allows only ONE sem wait per instruction. Tile's
            # wait elision relies on per-engine program order, which the
            # scheduler may permute. So: (a) pin every engine's stream to
            # emission order with no_sync edges, (b) warm each engine with
            # ops that absorb foreign sems one at a time, (c) per iteration,
            # absorber ops pick up semaphores so every real instruction
            # needs at most one new wait.
            prev = {}

            def chain(eng, inst, *sync_deps):
                for d in sync_deps:
                    add_dep_helper(inst.ins, d.ins, True, "warm")
                if eng in prev:
                    add_dep_helper(inst.ins, prev[eng].ins, False, "order")
                prev[eng] = inst
                return inst

            def pe(inst, *d):
                return chain("pe", inst, *d)

            def dve(inst, *d):
                return chain("dve", inst, *d)

            def act(inst, *d):
                return chain("act", inst, *d)

            def pool(inst, *d):
                return chain("pool", inst, *d)

            # engine warmups: absorb one foreign semaphore per instruction
            dve(nc.vector.tensor_copy(SCRD[0:32, 0:4], CONSTF[0:32, 0:4]),
                dma2)
            dve(nc.vector.tensor_scalar_mul(PRES_A[:, :], TQ_A, -1.0))
            dve(nc.vector.tensor_scalar_mul(PRES_B[:, :], TQ_B, -1.0))
            dve(nc.vector.tensor_copy(YP[:, :], CONSTF[:, C_YP:CF_W]))
            pool(nc.gpsimd.tensor_copy(SCRP[0:32, 4:8], CONSTF[0:32, 4:8]),
                 dma2)
            act(nc.scalar.copy(SCRA[0:32, 8:12], IDENT[0:32, 0:4]),
                *pool_insts)
            act(nc.scalar.copy(UFM9[0:32, 0:64], ONES32[:, :]),
                *dve_insts)
            act(nc.scalar.copy(YFMb[0][:, :], ZER128[:, 0:64]))
            act(nc.scalar.copy(YFMb[1][:, :], ZER128[:, 64:128]))
            act(nc.scalar.copy(YFMb[2][:, :], ZER128[:, 128:192]))
            pe(nc.tensor.transpose(G_A[0:64, 0:64], IDENT[0:64, 0:64],
                                   IDENT[0:64, 0:64]),
               *pool_insts)
            pe(nc.tensor.transpose(G_A[0:64, 0:64], U_A[:, 0:64],
                                   IDENT[0:64, 0:64]),
               *dve_insts)
            pe(nc.tensor.matmul(G_A[0:64, 0:64], KS2[0:128, 0:64],
                                KA1[0:128, 0:64], start=True, stop=True))

            for _it in range(ITERS):
                # ACT absorbers: a1 waits on the last ACT op of the previous
                # iteration (the YFM copy); a2 waits on a1's completion.
                act(nc.scalar.copy(SCRA[0:32, 0:4], YFMb[2][0:32, 0:4]))
                act(nc.scalar.copy(SCRA[0:32, 4:8], SCRA[0:32, 0:4]))

                # ---- MM1: G = tau*(y @ K), A half then B half ----
                for ci in range(3):
                    rw = 128 if ci < 2 else 32
                    pe(nc.tensor.matmul(
                        G_A[:, :], YFMb[ci][0:rw, 0:64],
                        KA1[0:rw, FPAD * ci:FPAD * ci + FA],
                        start=(ci == 0), stop=(ci == 2)))
                for (n0, nw) in ((512, 512), (1024, 256)):
                    for ci in range(3):
                        rw = 128 if ci < 2 else 32
                        pe(nc.tensor.matmul(
                            G_B[:, n0 - 512:n0 - 512 + nw],
                            YFMb[ci][0:rw, 0:64],
                            KA1[0:rw, FPAD * ci + n0:FPAD * ci + n0 + nw],
                            start=(ci == 0), stop=(ci == 2)))

                # ---- u update, A then B (DVE), pres on GPSIMD ----
                dve(nc.vector.tensor_copy(SCRD[0:32, 0:2], PRES_A[0:32, 0:2]))
                dve(nc.vector.scalar_tensor_tensor(
                    TMP_A[:, :], G_A[:, :], -1.0, PRES_A[:, :],
                    op.mult, op.add))
                dve(nc.vector.tensor_scalar(
                    U_A[:, :], TMP_A[:, :], 0.0, 1.0, op.max, op.min))
                dve(nc.vector.tensor_copy(SCRD[0:32, 2:4], PRES_B[0:32, 0:2]))
                dve(nc.vector.scalar_tensor_tensor(
                    TMP_B[:, :], G_B[:, 0:FB], -1.0, PRES_B[:, :],
                    op.mult, op.add))
                dve(nc.vector.tensor_scalar(
                    U_B[:, :], TMP_B[:, :], 0.0, 1.0, op.max, op.min))
                pool(nc.gpsimd.tensor_copy(SCRP[0:32, 2:4], SCRP[0:32, 0:2]))
                pool(nc.gpsimd.tensor_sub(PRES_A[:, :], U_A[:, :], TQ_A))
                pool(nc.gpsimd.tensor_sub(PRES_B[:, :], U_B[:, :], TQ_B))
                pool(nc.gpsimd.tensor_copy(SCRP[0:32, 0:2], PRES_B[0:32, 0:2]))

                # ---- transpose u to feat-major; MM2 accumulates P ----
                for c in range(4):        # chunks 0..3 from U_A
                    pe(nc.tensor.transpose(
                        TPX[:, 64 * c:64 * c + 64],
                        U_A[:, 128 * c:128 * c + 128], IDENT[0:64, 0:64]))
                act(nc.scalar.copy(UFM0[:, :], TPX[:, :]))
                # absorber: pick up ufm0's completion so the later TPX
                # read-read serializer deps (chunks 8/9) are pre-covered
                act(nc.scalar.copy(SCRA[0:32, 8:12], UFM0[0:32, 0:4]))
                for c in range(4):
                    pe(nc.tensor.matmul(
                        P[:, :], UFM0[0:128, 64 * c:64 * c + 64],
                        KS2[0:128, YR * c:YR * c + YR],
                        start=(c == 0), stop=False,
                        skip_group_check=True))
                # chunks 4..9 from U_B, grouped (2 transposes -> copy ->
                # 2 matmuls) so MM2 starts as soon as each pair lands
                def t1(c, dst, col):
                    cw = 128 if c < 9 else LAST_CW
                    pe(nc.tensor.transpose(
                        dst[0:cw, col:col + 64],
                        U_B[:, 128 * (c - 4):128 * (c - 4) + cw],
                        IDENT[0:64, 0:64]))

                def mm2(c, tile, col, stop=False):
                    rows = 128 if c < 9 else LAST_CW + 1
                    pe(nc.tensor.matmul(
                        P[:, :], tile[0:rows, col:col + 64],
                        KS2[0:rows, YR * c:YR * c + YR],
                        start=False, stop=stop,
                        skip_group_check=True))

                t1(4, TP45, 0)
                t1(5, TP45, 64)
                t1(6, TP67, 0)
                t1(7, TP67, 64)
                t1(8, TPX, 0)
                t1(9, TPX, 64)
                act(nc.scalar.copy(UFM45[:, :], TP45[:, :]))
                act(nc.scalar.copy(UFM67[:, :], TP67[:, :]))
                act(nc.scalar.copy(UFM8[:, :], TPX[:, 0:64]))
                act(nc.scalar.copy(UFM9[0:LAST_CW, 0:64],
                                   TPX[0:LAST_CW, 64:128]))
                mm2(4, UFM45, 0)
                mm2(5, UFM45, 64)
                mm2(6, UFM67, 0)
                mm2(7, UFM67, 64)
                mm2(8, UFM8, 0)
                mm2(9, UFM9, 0, stop=True)

                # ---- y update: y = relu(YP + 2P);  YP' = y - P ----
                dve(nc.vector.scalar_tensor_tensor(
                    T3[:, :], P[:, :], 2.0, YP[:, :], op.mult, op.add))
                dve(nc.vector.tensor_scalar_max(YBM[:, :], T3[:, :], 0.0))
                # ---- transpose y to feat-major ----
                for ci in range(3):
                    r0 = 128 * ci
                    rw = min(128, YR - r0)
                    pe(nc.tensor.transpose(
                        TY[0:rw, 64 * ci:64 * ci + 64], YBM[:, r0:r0 + rw],
                        IDENT[0:64, 0:64]))
                for ci in range(3):
                    rw = min(128, YR - 128 * ci)
                    act(nc.scalar.copy(
                        YFMb[ci][0:rw, 0:64],
                        TY[0:rw, 64 * ci:64 * ci + 64]))
                # off-critical: YP for next iter (reads P psum, so DVE)
                dve(nc.vector.scalar_tensor_tensor(
                    YP[:, :], P[:, :], -1.0, YBM[:, :], op.mult, op.add))

            zdma = nc.sync.dma_start(d_z[:, :], U_B[:, FB - N:FB])
            # tail fence: the framework drain waits on every proc, but the
            # ISA allows one wait per instruction — absorb them one at a
            # time with SP nops so the drain's own waits are elided.
            for d in (dma1, dma2, prev["pool"], prev["act"], prev["pe"],
                      prev["dve"], zdma):
                nn = nc.sync.nop()
                add_dep_helper(nn.ins, d.ins, True, "tail fence")
    return nc


LAST_RESULT = None


def kernel(weights, A, A_pos, b, _trace=False):
    weights = np.asarray(weights, np.float32)
    A = np.asarray(A, np.float32)
    A_pos = np.asarray(A_pos, np.float32)
    b = np.asarray(b, np.float32)

    K, h, tau = _build_constants(A, A_pos, b)
    ka1, ks2 = _host_tiles(K, h, tau)
    yp0 = np.broadcast_to(tau * h, (64, YR)).astype(np.float32).copy()

    nc = _build_bass()

    in_maps = []
    for core in range(N_CORES):
        w_shard = weights[core * BS:(core + 1) * BS].reshape(BS, N)
        tq = _per_core_tiles(w_shard, tau)
        cr, cf = _pack_consts(ka1, ks2, tq, yp0)
        in_maps.append({"constsr": cr, "constsf": cf})

    from concourse.bass_utils import run_bass_kernel_spmd
    res = run_bass_kernel_spmd(nc, in_maps, core_ids=list(range(N_CORES)),
                               trace=_trace)
    global LAST_RESULT
    LAST_RESULT = res
    outs = [np.asarray(res.results[c]["z"]) for c in range(N_CORES)]
    z = np.concatenate(outs, axis=0).reshape(B, N_GRID, N_GRID)
    return z.astype(np.float32)


if __name__ == "__main__":
    rng = np.random.default_rng(0)
    w = rng.random((B, N_GRID, N_GRID), np.float32)
    # smoke build only
    _build_bass()
    print("bass build OK")



# revision 26
# speedup vs baseline: 1.4507x; 1.1377x over previous
"""Trainium2 Bass kernel for nn_CvxDifflayer (batched PDHG LP solver).

Math (per batch row b):
    u_{k+1} = clip(u_k - tau*(q + y_k @ K), 0, 1)
    ubar    = 2*u_{k+1} - u_k
    y_{k+1} = relu(y_k + sigma*(ubar @ K.T - h))
    out z   = u_300[:, V:]  reshaped (12, 12)

Device reformulation (exact, per 64-batch shard):
    G_k  = tau*q + tau*(y_k @ K)        MM1: lhsT = Y feat-major, rhs = tau*K
    u    = clip(pres_k - G'_k)          where pres_k = u_k - tau*q, G' = tau*(yK)
    pres = u - tau*q                    (GPSIMD, off critical path)
    P_k  = sigma*(u @ K.T) - sigma*h    MM2: lhsT = u feat-major (PE transposes),
                                        h folded via constant ones-row in lhsT
    y    = relu(y + 2*P_k - P_{k-1})    (P_{-1} = -sigma*h since u_0 = 0)

Layouts per core (batch shard Bs=64):
    U     [128, 578]  batch-major fold: row 64*hf+b, col j = feature 578*hf+j
    UFM   [128, 640]  feat-major: chunk c cols [128c:128c+128); within chunk,
                      col 64*h+b = batch b of fold-half h; rows = fold-col
                      128c+i; chunk 4 rows 0:66 (+ row 66 = ones for h-fold)
    YBM   [64, 288]   batch-major y
    YFM   [128, 192]  feat-major y: block ci cols [64ci:64ci+64) = batch,
                      rows = y-row 128ci+i
"""

import sys

for _p in ("/opt/trn_rl_repo", "/opt/pypackages"):
    if _p not in sys.path:
        sys.path.insert(0, _p)

import numpy as np

N_GRID = 12
N = 144          # nodes
V = 1012         # directed edges
F = V + N        # 1156 primal vars
YR = 2 * N       # 288 dual vars
B = 512
BS = 64          # batch per core
N_CORES = 8
ITERS = 300
FPAD = 1280      # MM1 free dim padded so all chunks >= 256 (fp32r rate)
FM_CHUNKS = 10   # ceil(1156/128)
LAST_CW = F - 9 * 128      # 4


def _build_constants(A, A_pos, b):
    K = np.zeros((YR, F), np.float32)
    K[:N, :V] = A
    K[N:, :V] = A_pos
    K[N:, V:] = -np.eye(N, dtype=np.float32)
    h = np.concatenate([b.astype(np.float32), np.zeros(N, np.float32)])
    Kn = np.float32(np.sqrt(np.abs(K).sum(0).max() * np.abs(K).sum(1).max()))
    tau = np.float32(0.9) / Kn
    return K, h, tau


def _host_tiles(K, h, tau):
    """Constant SBUF images shared by all cores."""
    sigma = tau
    tauK = (tau * K).astype(np.float32)          # (288, 1156)
    sigK = (sigma * K).astype(np.float32)

    # KA1: MM1 rhs, 3 contraction chunks side by side, free dim padded to
    # FPAD so every matmul free-chunk is >=256 (full fp32r rate)
    ka1 = np.zeros((128, 3 * FPAD), np.float32)
    for r in range(3):
        r0 = 128 * r
        rw = min(128, YR - r0)
        ka1[:rw, FPAD * r:FPAD * r + F] = tauK[r0:r0 + rw, :]

    # KS2: MM2 rhs, 10 feat chunks of [rows, 288] side by side
    ks2 = np.zeros((128, 10 * YR), np.float32)
    for c in range(FM_CHUNKS):
        cw = 128 if c < 9 else LAST_CW
        f0 = 128 * c
        ks2[:cw, YR * c:YR * c + YR] = sigK[:, f0:f0 + cw].T
        if c == 9:
            ks2[LAST_CW, YR * c:YR * c + YR] = -sigma * h  # ones-row fold
    # z-trimmed MM2: chunk 8 region is unread; its row 0 carries -sigma*h
    # for the 1-row h-fold matmul (lhsT = ones row)
    ks2[0, YR * 8:YR * 8 + YR] = -sigma * h
    return ka1, ks2


def _per_core_tiles(w_shard, tau):
    """tq for one 64-row batch shard; w_shard (64, 144)."""
    tq = np.zeros((64, F), np.float32)
    tq[:, V:] = tau * w_shard
    return tq


# constsr layout (f32r, read-only): ka1 | ks2
C_KA1 = 0
C_KS2 = C_KA1 + 3 * FPAD
CR_W = C_KS2 + 10 * YR
# constsf layout (f32, read-only): tq | yp0  (yp0 = +sigma*h)
C_TQ = 0
C_YP = C_TQ + F
CF_W = C_YP + YR

FA = 512          # feature split: A = 0:512, B = 512:1156
FB = F - FA       # 644


def _pack_consts(ka1, ks2, tq, yp0):
    cr = np.zeros((128, CR_W), np.float32)
    cr[:, C_KA1:C_KS2] = ka1
    cr[:, C_KS2:CR_W] = ks2
    cf = np.zeros((64, CF_W), np.float32)
    cf[:, C_TQ:C_YP] = tq
    cf[:, C_YP:CF_W] = yp0
    return cr, cf


TAUS = np.zeros(1, np.float32)


def _build_bass():
    from concourse import bass, mybir
    from concourse.tile import TileContext
    from concourse.tile_rust import add_dep_helper
    from concourse.mybir import AluOpType as op

    f32 = mybir.dt.float32
    f32r = mybir.dt.float32r

    nc = bass.Bass()
    d_cr = nc.dram_tensor("constsr", (128, CR_W), f32r, kind="ExternalInput")
    d_cf = nc.dram_tensor("constsf", (64, CF_W), f32, kind="ExternalInput")
    d_z = nc.dram_tensor("z", (64, N), f32, kind="ExternalOutput")

    with TileContext(nc) as tc:
        with (
            tc.tile_pool(name="state", bufs=1) as sp,
            tc.tile_pool(name="psA", bufs=1, space="PSUM") as psA,
            tc.tile_pool(name="psB", bufs=1, space="PSUM") as psB,
            tc.tile_pool(name="psP", bufs=1, space="PSUM") as psP,
            tc.tile_pool(name="psT0", bufs=1, space="PSUM") as psT0,
            tc.tile_pool(name="psT1", bufs=1, space="PSUM") as psT1,
            tc.tile_pool(name="psTY", bufs=1, space="PSUM") as psTY,
        ):
            CONSTR = sp.tile([128, CR_W], f32r)
            CONSTF = sp.tile([64, CF_W], f32)
            KA1 = CONSTR[:, C_KA1:C_KS2]
            KS2 = CONSTR[:, C_KS2:CR_W]
            TQ_A = CONSTF[:, C_TQ:C_TQ + FA]
            TQ_B = CONSTF[:, C_TQ + FA:C_YP]
            U_A = sp.tile([64, FA], f32)
            U_B = sp.tile([64, FB], f32)
            TMP_A = sp.tile([64, FA], f32)
            TMP_B = sp.tile([64, FB], f32)
            PRES_A = sp.tile([64, FA], f32)
            PRES_B = sp.tile([64, FB], f32)
            YP = sp.tile([64, YR], f32)
            YBM = sp.tile([64, YR], f32)
            T3 = sp.tile([64, YR], f32)
            # per-engine scratch tiles (separate so absorber ops never
            # create cross-engine tile deps)
            SCRD = sp.tile([32, 8], f32)
            SCRA = sp.tile([32, 12], f32)
            SCRP = sp.tile([32, 8], f32)
            ONES32 = sp.tile([32, 64], f32)
            ZER128 = sp.tile([128, 192], f32)
            # feat-major u in two wide tiles; ones-row for the h-fold at
            # row LAST_CW of the chunk-9 column block of UFM1
            UFM0 = sp.tile([128, 256], f32r)   # chunks 0..3
            UFM45 = sp.tile([128, 128], f32r)  # chunks 4,5
            UFM67 = sp.tile([128, 128], f32r)  # chunks 6,7
            UFM8 = sp.tile([128, 64], f32r)    # chunk 8
            UFM9 = sp.tile([32, 64], f32r)     # chunk 9 + ones row
            YFMb = []
            for c in range(3):
                yfm_c = sp.tile([128, 64], f32r, tag=f"yfm{c}")
                YFMb.append(yfm_c)
            IDENT = sp.tile([128, 128], f32)

            dma1 = nc.sync.dma_start(CONSTR[:, :], d_cr[:, :])
            dma2 = nc.sync.dma_start(CONSTF[:, :], d_cf[:, :])

            pool_insts = [
                nc.gpsimd.memset(IDENT[:, :], 0.0),
                nc.gpsimd.affine_select(
                    out=IDENT[:, :], in_=IDENT[:, :],
                    compare_op=mybir.AluOpType.not_equal, fill=1.0, base=0,
                    pattern=[[-1, 128]], channel_multiplier=1),
            ]
            dve_insts = [
                nc.vector.memset(U_A[:, :], 0.0),
                nc.vector.memset(U_B[:, :], 0.0),
                nc.vector.memset(ONES32[:, :], 1.0),
                nc.vector.memset(ZER128[:, :], 0.0),
                nc.vector.memset(YBM[:, :], 0.0),
            ]

            G_A = psA.tile([64, FA], f32)
            G_B = psB.tile([64, 768], f32)
            P = psP.tile([64, YR], f32)
            TPX = psT0.tile([128, 256], f32)   # chunks 0..3, then 8..9
            TP45 = psT1.tile([128, 128], f32)
            TP67 = psT1.tile([128, 128], f32, tag="tp67")
            TY = psTY.tile([128, 192], f32)

            # This target allows only ONE sem wait per instruction. Tile's
            # wait elision relies on per-engine program order, which the
            # scheduler may permute. So: (a) pin every engine's stream to
            # emission order with no_sync edges, (b) warm each engine with
            # ops that absorb foreign sems one at a time, (c) per iteration,
            # absorber ops pick up semaphores so every real instruction
            # needs at most one new wait.
            prev = {}

            def chain(eng, inst, *sync_deps):
                for d in sync_deps:
                    add_dep_helper(inst.ins, d.ins, True, "warm")
                if eng in prev:
                    add_dep_helper(inst.ins, prev[eng].ins, False, "order")
                prev[eng] = inst
                return inst

            def pe(inst, *d):
                return chain("pe", inst, *d)

            def dve(inst, *d):
                return chain("dve", inst, *d)

            def act(inst, *d):
                return chain("act", inst, *d)

            def pool(inst, *d):
                return chain("pool", inst, *d)

            # engine warmups: absorb one foreign semaphore per instruction
            dve(nc.vector.tensor_copy(SCRD[0:32, 0:4], CONSTF[0:32, 0:4]),
                dma2)
            dve(nc.vector.tensor_scalar_mul(PRES_A[:, :], TQ_A, -1.0))
            dve(nc.vector.tensor_scalar_mul(PRES_B[:, :], TQ_B, -1.0))
            dve(nc.vector.tensor_copy(YP[:, :], CONSTF[:, C_YP:CF_W]))
            pool(nc.gpsimd.tensor_copy(SCRP[0:32, 4:8], CONSTF[0:32, 4:8]),
                 dma2)
            act(nc.scalar.copy(SCRA[0:32, 8:12], IDENT[0:32, 0:4]),
                *pool_insts)
            act(nc.scalar.copy(UFM9[0:32, 0:64], ONES32[:, :]),
                *dve_insts)
            act(nc.scalar.copy(YFMb[0][:, :], ZER128[:, 0:64]))
            act(nc.scalar.copy(YFMb[1][:, :], ZER128[:, 64:128]))
            act(nc.scalar.copy(YFMb[2][:, :], ZER128[:, 128:192]))
            pe(nc.tensor.transpose(G_A[0:64, 0:64], IDENT[0:64, 0:64],
                                   IDENT[0:64, 0:64]),
               *pool_insts)
            pe(nc.tensor.transpose(G_A[0:64, 0:64], U_A[:, 0:64],
                                   IDENT[0:64, 0:64]),
               *dve_insts)
            pe(nc.tensor.matmul(G_A[0:64, 0:64], KS2[0:128, 0:64],
                                KA1[0:128, 0:64], start=True, stop=True))

            for _it in range(ITERS):
                # ACT absorbers: a1 waits on the last ACT op of the previous
                # iteration (the YFM copy); a2 waits on a1's completion.
                act(nc.scalar.copy(SCRA[0:32, 0:4], YFMb[2][0:32, 0:4]))
                act(nc.scalar.copy(SCRA[0:32, 4:8], SCRA[0:32, 0:4]))

                # ---- MM1: G = tau*(y @ K), A half then B half ----
                for ci in range(3):
                    rw = 128 if ci < 2 else 32
                    pe(nc.tensor.matmul(
                        G_A[:, :], YFMb[ci][0:rw, 0:64],
                        KA1[0:rw, FPAD * ci:FPAD * ci + FA],
                        start=(ci == 0), stop=(ci == 2)))
                for ci in range(3):
                    rw = 128 if ci < 2 else 32
                    pe(nc.tensor.matmul(
                        G_B[:, 0:500], YFMb[ci][0:rw, 0:64],
                        KA1[0:rw, FPAD * ci + 512:FPAD * ci + 1012],
                        start=(ci == 0), stop=(ci == 2)))

                # ---- u update, A then B (DVE), pres on GPSIMD ----
                dve(nc.vector.tensor_copy(SCRD[0:32, 0:2], PRES_A[0:32, 0:2]))
                dve(nc.vector.scalar_tensor_tensor(
                    TMP_A[:, :], G_A[:, :], -1.0, PRES_A[:, :],
                    op.mult, op.add))
                dve(nc.vector.tensor_scalar(
                    U_A[:, :], TMP_A[:, :], 0.0, 1.0, op.max, op.min))
                dve(nc.vector.tensor_copy(SCRD[0:32, 2:4], PRES_B[0:32, 0:2]))
                dve(nc.vector.scalar_tensor_tensor(
                    TMP_B[:, 0:500], G_B[:, 0:500], -1.0, PRES_B[:, 0:500],
                    op.mult, op.add))
                dve(nc.vector.scalar_tensor_tensor(
                    TMP_B[:, 500:FB], YBM[:, N:YR], float(TAUS[0]),
                    PRES_B[:, 500:FB], op.mult, op.add))
                dve(nc.vector.tensor_scalar(
                    U_B[:, :], TMP_B[:, :], 0.0, 1.0, op.max, op.min))
                pool(nc.gpsimd.tensor_copy(SCRP[0:32, 2:4], SCRP[0:32, 0:2]))
                pool(nc.gpsimd.tensor_sub(PRES_A[:, :], U_A[:, :], TQ_A))
                pool(nc.gpsimd.tensor_sub(PRES_B[:, :], U_B[:, :], TQ_B))
                pool(nc.gpsimd.tensor_copy(SCRP[0:32, 0:2], PRES_B[0:32, 0:2]))

                # ---- transpose u to feat-major; MM2 accumulates P ----
                for c in range(4):        # chunks 0..3 from U_A
                    pe(nc.tensor.transpose(
                        TPX[:, 64 * c:64 * c + 64],
                        U_A[:, 128 * c:128 * c + 128], IDENT[0:64, 0:64]))
                act(nc.scalar.copy(UFM0[:, :], TPX[:, :]))
                # absorber: pick up ufm0's completion so the later TPX
                # read-read serializer deps (chunks 8/9) are pre-covered
                act(nc.scalar.copy(SCRA[0:32, 8:12], UFM0[0:32, 0:4]))
                pe(nc.tensor.matmul(
                    P[:, :], UFM9[0:1, 0:64], KS2[0:1, YR * 8:YR * 9],
                    start=True, stop=False, skip_group_check=True))
                for c in range(4):
                    pe(nc.tensor.matmul(
                        P[:, :], UFM0[0:128, 64 * c:64 * c + 64],
                        KS2[0:128, YR * c:YR * c + YR],
                        start=False, stop=False,
                        skip_group_check=True))
                # chunks 4..9 from U_B, grouped (2 transposes -> copy ->
                # 2 matmuls) so MM2 starts as soon as each pair lands
                def t1(c, dst, col):
                    cw = 128 if c < 9 else LAST_CW
                    pe(nc.tensor.transpose(
                        dst[0:cw, col:col + 64],
                        U_B[:, 128 * (c - 4):128 * (c - 4) + cw],
                        IDENT[0:64, 0:64]))

                def mm2(c, tile, col, stop=False):
                    rows = 128 if c < 7 else 116
                    pe(nc.tensor.matmul(
                        P[:, :], tile[0:rows, col:col + 64],
                        KS2[0:rows, YR * c:YR * c + YR],
                        start=False, stop=stop,
                        skip_group_check=True))

                t1(4, TP45, 0)
                t1(5, TP45, 64)
                t1(6, TP67, 0)
                t1(7, TP67, 64)
                act(nc.scalar.copy(UFM45[:, :], TP45[:, :]))
                act(nc.scalar.copy(UFM67[:, :], TP67[:, :]))
                mm2(4, UFM45, 0)
                mm2(5, UFM45, 64)
                mm2(6, UFM67, 0)
                mm2(7, UFM67, 64, stop=True)

                # ---- y update: y = relu(YP + 2P);  YP' = y - P ----
                dve(nc.vector.scalar_tensor_tensor(
                    T3[:, :], P[:, :], 2.0, YP[:, :], op.mult, op.add))
                dve(nc.vector.scalar_tensor_tensor(
                    T3[:, N:YR], U_B[:, 500:FB], -2.0 * float(TAUS[0]),
                    T3[:, N:YR], op.mult, op.add))
                dve(nc.vector.tensor_scalar_max(YBM[:, :], T3[:, :], 0.0))
                # ---- transpose y to feat-major ----
                for ci in range(3):
                    r0 = 128 * ci
                    rw = min(128, YR - r0)
                    pe(nc.tensor.transpose(
                        TY[0:rw, 64 * ci:64 * ci + 64], YBM[:, r0:r0 + rw],
                        IDENT[0:64, 0:64]))
                for ci in range(3):
                    rw = min(128, YR - 128 * ci)
                    act(nc.scalar.copy(
                        YFMb[ci][0:rw, 0:64],
                        TY[0:rw, 64 * ci:64 * ci + 64]))
                # off-critical: YP for next iter (reads P psum, so DVE)
                dve(nc.vector.scalar_tensor_tensor(
                    YP[:, :], P[:, :], -1.0, YBM[:, :], op.mult, op.add))
                dve(nc.vector.scalar_tensor_tensor(
                    YP[:, N:YR], U_B[:, 500:FB], float(TAUS[0]),
                    YP[:, N:YR], op.mult, op.add))

            zdma = nc.sync.dma_start(d_z[:, :], U_B[:, FB - N:FB])
            # tail fence: the framework drain waits on every proc, but the
            # ISA allows one wait per instruction — absorb them one at a
            # time with SP nops so the drain's own waits are elided.
            for d in (dma1, dma2, prev["pool"], prev["act"], prev["pe"],
                      prev["dve"], zdma):
                nn = nc.sync.nop()
                add_dep_helper(nn.ins, d.ins, True, "tail fence")
    return nc


LAST_RESULT = None


def kernel(weights, A, A_pos, b, _trace=False):
    weights = np.asarray(weights, np.float32)
    A = np.asarray(A, np.float32)
    A_pos = np.asarray(A_pos, np.float32)
    b = np.asarray(b, np.float32)

    K, h, tau = _build_constants(A, A_pos, b)
    TAUS[0] = tau
    ka1, ks2 = _host_tiles(K, h, tau)
    yp0 = np.broadcast_to(tau * h, (64, YR)).astype(np.float32).copy()

    nc = _build_bass()

    in_maps = []
    for core in range(N_CORES):
        w_shard = weights[core * BS:(core + 1) * BS].reshape(BS, N)
        tq = _per_core_tiles(w_shard, tau)
        cr, cf = _pack_consts(ka1, ks2, tq, yp0)
        in_maps.append({"constsr": cr, "constsf": cf})

    from concourse.bass_utils import run_bass_kernel_spmd
    res = run_bass_kernel_spmd(nc, in_maps, core_ids=list(range(N_CORES)),
                               trace=_trace)
    global LAST_RESULT
    LAST_RESULT = res
    outs = [np.asarray(res.results[c]["z"]) for c in range(N_CORES)]
    z = np.concatenate(outs, axis=0).reshape(B, N_GRID, N_GRID)
    return z.astype(np.float32)


if __name__ == "__main__":
    TAUS[0] = 0.1
    rng = np.random.default_rng(0)
    w = rng.random((B, N_GRID, N_GRID), np.float32)
    # smoke build only
    _build_bass()
    print("bass build OK")



# revision 28
# speedup vs baseline: 1.8443x; 1.2713x over previous
"""Trainium2 Bass kernel for nn_CvxDifflayer (batched PDHG LP solver).

Math (per batch row b):
    u_{k+1} = clip(u_k - tau*(q + y_k @ K), 0, 1)
    ubar    = 2*u_{k+1} - u_k
    y_{k+1} = relu(y_k + sigma*(ubar @ K.T - h))
    out z   = u_300[:, V:]  reshaped (12, 12)

Device reformulation (exact, per 64-batch shard):
    G_k  = tau*q + tau*(y_k @ K)        MM1: lhsT = Y feat-major, rhs = tau*K
    u    = clip(pres_k - G'_k)          where pres_k = u_k - tau*q, G' = tau*(yK)
    pres = u - tau*q                    (GPSIMD, off critical path)
    P_k  = sigma*(u @ K.T) - sigma*h    MM2: lhsT = u feat-major (PE transposes),
                                        h folded via constant ones-row in lhsT
    y    = relu(y + 2*P_k - P_{k-1})    (P_{-1} = -sigma*h since u_0 = 0)

Layouts per core (batch shard Bs=64):
    U     [128, 578]  batch-major fold: row 64*hf+b, col j = feature 578*hf+j
    UFM   [128, 640]  feat-major: chunk c cols [128c:128c+128); within chunk,
                      col 64*h+b = batch b of fold-half h; rows = fold-col
                      128c+i; chunk 4 rows 0:66 (+ row 66 = ones for h-fold)
    YBM   [64, 288]   batch-major y
    YFM   [128, 192]  feat-major y: block ci cols [64ci:64ci+64) = batch,
                      rows = y-row 128ci+i
"""

import sys

for _p in ("/opt/trn_rl_repo", "/opt/pypackages"):
    if _p not in sys.path:
        sys.path.insert(0, _p)

import numpy as np

N_GRID = 12
N = 144          # nodes
V = 1012         # directed edges
F = V + N        # 1156 primal vars
YR = 2 * N       # 288 dual vars
B = 512
BS = 64          # batch per core
N_CORES = 8
ITERS = 300
FPAD = 1280      # MM1 free dim padded so all chunks >= 256 (fp32r rate)
FM_CHUNKS = 10   # ceil(1156/128)
LAST_CW = F - 9 * 128      # 4


def _tables():
    offs = [(p, q) for p in (-1, 0, 1) for q in (-1, 0, 1) if (p, q) != (0, 0)]
    es = []
    for i in range(N_GRID):
        for j in range(N_GRID):
            for p, q in offs:
                ii, jj = i + p, j + q
                if 0 <= ii < N_GRID and 0 <= jj < N_GRID:
                    es.append((i * N_GRID + j, ii * N_GRID + jj))

    def nb(node):
        return (node // N_GRID) // 4

    keys = [(min(nb(s), nb(t)), max(nb(s), nb(t)), s) for (s, t) in es]
    order = sorted(range(V), key=lambda e: keys[e])
    perm_e = np.array(order, dtype=np.int64)
    segs = []
    st, cur = 0, keys[order[0]][:2]
    for j in range(1, V + 1):
        if j == V or keys[order[j]][:2] != cur:
            segs.append((cur[0], cur[1], st, j))
            if j < V:
                cur, st = keys[order[j]][:2], j
    ss = []
    for (b1, b2, s0, s1) in segs:
        if s0 < 512 < s1:
            ss += [(b1, b2, s0, 512), (b1, b2, 512, s1)]
        else:
            ss.append((b1, b2, s0, s1))
    mm1 = []
    for bank in (0, 1):
        for bd in range(3):
            for (b1, b2, s0, s1) in ss:
                if bd in (b1, b2) and ((s1 <= 512) == (bank == 0)):
                    mm1.append((bd, s0, s1, bd == b1))
    y_perm = np.empty(YR, dtype=np.int64)
    y_perm[0::2] = np.arange(N)
    y_perm[1::2] = N + np.arange(N)
    chunks = []
    for c0 in range(0, V, 128):
        c1 = min(c0 + 128, V)
        nodes = set()
        for j in range(c0, c1):
            s, t = es[perm_e[j]]
            nodes.add(s)
            nodes.add(t)
        chunks.append((c0, c1, 2 * min(nodes), 2 * max(nodes) + 2))
    return perm_e, y_perm, mm1, chunks


PERM_E, Y_PERM, MM1_TAB, MM2_CHUNKS = _tables()
BAND_COL0 = [0, 292, 652]
KA1_OFF = [0, 360, 788]
KS2_OFF = np.cumsum(
    [0] + [hi - lo for (_, _, lo, hi) in MM2_CHUNKS]).tolist()


def _mm1_rhs_off(bd, c0):
    if bd == 1 and c0 >= 512:
        return 580 + (c0 - 512)
    if bd == 2 and c0 >= 720:
        return 856 + (c0 - 720)
    return KA1_OFF[bd] + (c0 - BAND_COL0[bd])


def _build_constants(A, A_pos, b):
    K = np.zeros((YR, F), np.float32)
    K[:N, :V] = A
    K[N:, :V] = A_pos
    K[N:, V:] = -np.eye(N, dtype=np.float32)
    h = np.concatenate([b.astype(np.float32), np.zeros(N, np.float32)])
    Kn = np.float32(np.sqrt(np.abs(K).sum(0).max() * np.abs(K).sum(1).max()))
    tau = np.float32(0.9) / Kn
    return K, h, tau


def _host_tiles(K, h, tau):
    """Banded constant SBUF images (edge/y permuted) shared by all cores."""
    sigma = tau
    Kx = K[:, :V][np.ix_(Y_PERM, PERM_E)]        # (288, 1012) permuted
    hp = h[Y_PERM]
    tKx = (tau * Kx).astype(np.float32)
    sKx = (sigma * Kx).astype(np.float32)

    # KA1: MM1 rhs band blocks: band0 x[0:360) @0, band1 x[292:512) @360,
    # band1 x[512:720) @580, band2 x[652:720) @788, band2 x[720:1012) @856
    ka1 = np.zeros((128, KA1_W), np.float32)
    for bd, r0, c0, w, off in ((0, 0, 0, 360, 0), (1, 96, 292, 220, 360),
                               (1, 96, 512, 208, 580), (2, 192, 652, 68, 788),
                               (2, 192, 720, 292, 856)):
        ka1[0:96, off:off + w] = tKx[r0:r0 + 96, c0:c0 + w]

    # KS2: MM2 span blocks + h-row at [KS2_H:KS2_H+288) row 0
    ks2 = np.zeros((128, KS2_W), np.float32)
    for ci, (c0, c1, ylo, yhi) in enumerate(MM2_CHUNKS):
        ks2[0:c1 - c0, KS2_OFF[ci]:KS2_OFF[ci + 1]] = sKx[ylo:yhi, c0:c1].T
    ks2[0, KS2_H:KS2_H + YR] = -sigma * hp
    return ka1, ks2


def _per_core_tiles(w_shard, tau):
    """tq for one 64-row batch shard; w_shard (64, 144)."""
    tq = np.zeros((64, F), np.float32)
    tq[:, V:] = tau * w_shard
    return tq


# constsr layout (f32r, read-only): ka1 | ks2
KA1_W = 1148
KS2_H = KS2_OFF[-1]
KS2_W = KS2_H + YR
C_KA1 = 0
C_KS2 = C_KA1 + KA1_W
CR_W = C_KS2 + KS2_W
# constsf layout (f32, read-only): tq | yp0  (yp0 = +sigma*h)
C_TQ = 0
C_YP = C_TQ + F
CF_W = C_YP + YR

FA = 512          # feature split: A = 0:512, B = 512:1156
FB = F - FA       # 644


def _pack_consts(ka1, ks2, tq, yp0):
    cr = np.zeros((128, CR_W), np.float32)
    cr[:, C_KA1:C_KS2] = ka1
    cr[:, C_KS2:CR_W] = ks2
    cf = np.zeros((64, CF_W), np.float32)
    cf[:, C_TQ:C_YP] = tq
    cf[:, C_YP:CF_W] = yp0
    return cr, cf


TAUS = np.zeros(1, np.float32)


def _build_bass():
    from concourse import bass, mybir
    from concourse.tile import TileContext
    from concourse.tile_rust import add_dep_helper
    from concourse.mybir import AluOpType as op

    f32 = mybir.dt.float32
    f32r = mybir.dt.float32r

    nc = bass.Bass()
    d_cr = nc.dram_tensor("constsr", (128, CR_W), f32r, kind="ExternalInput")
    d_cf = nc.dram_tensor("constsf", (64, CF_W), f32, kind="ExternalInput")
    d_z = nc.dram_tensor("z", (64, N), f32, kind="ExternalOutput")

    with TileContext(nc) as tc:
        with (
            tc.tile_pool(name="state", bufs=1) as sp,
            tc.tile_pool(name="psA", bufs=1, space="PSUM") as psA,
            tc.tile_pool(name="psB", bufs=1, space="PSUM") as psB,
            tc.tile_pool(name="psP", bufs=1, space="PSUM") as psP,
            tc.tile_pool(name="psT0", bufs=1, space="PSUM") as psT0,
            tc.tile_pool(name="psT1", bufs=1, space="PSUM") as psT1,
            tc.tile_pool(name="psTY", bufs=1, space="PSUM") as psTY,
        ):
            CONSTR = sp.tile([128, CR_W], f32r)
            CONSTF = sp.tile([64, CF_W], f32)
            KA1 = CONSTR[:, C_KA1:C_KS2]
            KS2 = CONSTR[:, C_KS2:CR_W]
            TQ_A = CONSTF[:, C_TQ:C_TQ + FA]
            TQ_B = CONSTF[:, C_TQ + FA:C_YP]
            U_A = sp.tile([64, FA], f32)
            U_B = sp.tile([64, FB], f32)
            TMP_A = sp.tile([64, FA], f32)
            TMP_B = sp.tile([64, FB], f32)
            PRES_A = sp.tile([64, FA], f32)
            PRES_B = sp.tile([64, FB], f32)
            YP = sp.tile([64, YR], f32)
            YBM = sp.tile([64, YR], f32)
            T3 = sp.tile([64, YR], f32)
            # per-engine scratch tiles (separate so absorber ops never
            # create cross-engine tile deps)
            SCRD = sp.tile([32, 8], f32)
            SCRA = sp.tile([32, 12], f32)
            SCRP = sp.tile([32, 8], f32)
            ONES32 = sp.tile([32, 64], f32)
            ZER128 = sp.tile([128, 192], f32)
            # feat-major u in two wide tiles; ones-row for the h-fold at
            # row LAST_CW of the chunk-9 column block of UFM1
            UFM0 = sp.tile([128, 256], f32r)   # chunks 0..3
            UFM45 = sp.tile([128, 128], f32r)  # chunks 4,5
            UFM67 = sp.tile([128, 128], f32r)  # chunks 6,7
            UFM8 = sp.tile([128, 64], f32r)    # chunk 8
            UFM9 = sp.tile([32, 64], f32r)     # chunk 9 + ones row
            YFMb = []
            for c in range(3):
                yfm_c = sp.tile([128, 64], f32r, tag=f"yfm{c}")
                YFMb.append(yfm_c)
            IDENT = sp.tile([128, 128], f32)

            dma1 = nc.sync.dma_start(CONSTR[:, :], d_cr[:, :])
            dma2 = nc.sync.dma_start(CONSTF[:, :], d_cf[:, :])

            pool_insts = [
                nc.gpsimd.memset(IDENT[:, :], 0.0),
                nc.gpsimd.affine_select(
                    out=IDENT[:, :], in_=IDENT[:, :],
                    compare_op=mybir.AluOpType.not_equal, fill=1.0, base=0,
                    pattern=[[-1, 128]], channel_multiplier=1),
            ]
            dve_insts = [
                nc.vector.memset(U_A[:, :], 0.0),
                nc.vector.memset(U_B[:, :], 0.0),
                nc.vector.memset(ONES32[:, :], 1.0),
                nc.vector.memset(ZER128[:, :], 0.0),
                nc.vector.memset(YBM[:, :], 0.0),
            ]

            G_A = psA.tile([64, FA], f32)
            G_B = psB.tile([64, 768], f32)
            P = psP.tile([64, YR], f32)
            TPX = psT0.tile([128, 256], f32)   # chunks 0..3, then 8..9
            TP45 = psT1.tile([128, 128], f32)
            TP67 = psT1.tile([128, 128], f32, tag="tp67")
            TY = psTY.tile([128, 192], f32)

            # This target allows only ONE sem wait per instruction. Tile's
            # wait elision relies on per-engine program order, which the
            # scheduler may permute. So: (a) pin every engine's stream to
            # emission order with no_sync edges, (b) warm each engine with
            # ops that absorb foreign sems one at a time, (c) per iteration,
            # absorber ops pick up semaphores so every real instruction
            # needs at most one new wait.
            prev = {}

            def chain(eng, inst, *sync_deps):
                for d in sync_deps:
                    add_dep_helper(inst.ins, d.ins, True, "warm")
                if eng in prev:
                    add_dep_helper(inst.ins, prev[eng].ins, False, "order")
                prev[eng] = inst
                return inst

            def pe(inst, *d):
                return chain("pe", inst, *d)

            def dve(inst, *d):
                return chain("dve", inst, *d)

            def act(inst, *d):
                return chain("act", inst, *d)

            def pool(inst, *d):
                return chain("pool", inst, *d)

            # engine warmups: absorb one foreign semaphore per instruction
            dve(nc.vector.tensor_copy(SCRD[0:32, 0:4], CONSTF[0:32, 0:4]),
                dma2)
            dve(nc.vector.tensor_scalar_mul(PRES_A[:, :], TQ_A, -1.0))
            dve(nc.vector.tensor_scalar_mul(PRES_B[:, :], TQ_B, -1.0))
            dve(nc.vector.tensor_copy(YP[:, :], CONSTF[:, C_YP:CF_W]))
            pool(nc.gpsimd.tensor_copy(SCRP[0:32, 4:8], CONSTF[0:32, 4:8]),
                 dma2)
            act(nc.scalar.copy(SCRA[0:32, 8:12], IDENT[0:32, 0:4]),
                *pool_insts)
            act(nc.scalar.copy(UFM9[0:32, 0:64], ONES32[:, :]),
                *dve_insts)
            act(nc.scalar.copy(YFMb[0][:, :], ZER128[:, 0:64]))
            act(nc.scalar.copy(YFMb[1][:, :], ZER128[:, 64:128]))
            act(nc.scalar.copy(YFMb[2][:, :], ZER128[:, 128:192]))
            pe(nc.tensor.transpose(G_A[0:64, 0:64], IDENT[0:64, 0:64],
                                   IDENT[0:64, 0:64]),
               *pool_insts)
            pe(nc.tensor.transpose(G_A[0:64, 0:64], U_A[:, 0:64],
                                   IDENT[0:64, 0:64]),
               *dve_insts)
            pe(nc.tensor.matmul(G_A[0:64, 0:64], KS2[0:128, 0:64],
                                KA1[0:128, 0:64], start=True, stop=True))

            for _it in range(ITERS):
                # ACT absorbers: a1 waits on the last ACT op of the previous
                # iteration (the YFM copy); a2 waits on a1's completion.
                act(nc.scalar.copy(SCRA[0:32, 0:4], YFMb[2][0:32, 0:4]))
                act(nc.scalar.copy(SCRA[0:32, 4:8], SCRA[0:32, 0:4]))

                # ---- MM1: G = tau*(y @ Kx), banded segments; closed
                # psum groups per segment (multi-group deps merge on the
                # PE sem for the DVE readers) ----
                from collections import Counter as _C
                segw = _C((c0, c1) for (_, c0, c1, _) in MM1_TAB)
                seen = _C()
                for (bd, c0, c1, first) in MM1_TAB:
                    seen[(c0, c1)] += 1
                    last = seen[(c0, c1)] == segw[(c0, c1)]
                    out = (G_A[:, c0:c1] if c1 <= 512
                           else G_B[:, c0 - 512:c1 - 512])
                    off = _mm1_rhs_off(bd, c0)
                    pe(nc.tensor.matmul(
                        out, YFMb[bd][0:96, 0:64],
                        KA1[0:96, off:off + (c1 - c0)],
                        start=first, stop=last, skip_group_check=True))

                # ---- u update, A then B (DVE), pres on GPSIMD ----
                dve(nc.vector.tensor_copy(SCRD[0:32, 0:2], PRES_A[0:32, 0:2]))
                dve(nc.vector.scalar_tensor_tensor(
                    TMP_A[:, :], G_A[:, :], -1.0, PRES_A[:, :],
                    op.mult, op.add))
                dve(nc.vector.tensor_scalar(
                    U_A[:, :], TMP_A[:, :], 0.0, 1.0, op.max, op.min))
                dve(nc.vector.tensor_copy(SCRD[0:32, 2:4], PRES_B[0:32, 0:2]))
                dve(nc.vector.scalar_tensor_tensor(
                    TMP_B[:, 0:500], G_B[:, 0:500], -1.0, PRES_B[:, 0:500],
                    op.mult, op.add))
                dve(nc.vector.scalar_tensor_tensor(
                    TMP_B[:, 500:FB], YBM[:, 1::2], float(TAUS[0]),
                    PRES_B[:, 500:FB], op.mult, op.add))
                dve(nc.vector.tensor_scalar(
                    U_B[:, :], TMP_B[:, :], 0.0, 1.0, op.max, op.min))
                pool(nc.gpsimd.tensor_copy(SCRP[0:32, 2:4], SCRP[0:32, 0:2]))
                pool(nc.gpsimd.tensor_sub(PRES_A[:, :], U_A[:, :], TQ_A))
                pool(nc.gpsimd.tensor_sub(PRES_B[:, :], U_B[:, :], TQ_B))
                pool(nc.gpsimd.tensor_copy(SCRP[0:32, 0:2], PRES_B[0:32, 0:2]))

                # ---- transpose u to feat-major; MM2 accumulates P ----
                for c in range(4):        # chunks 0..3 from U_A
                    pe(nc.tensor.transpose(
                        TPX[:, 64 * c:64 * c + 64],
                        U_A[:, 128 * c:128 * c + 128], IDENT[0:64, 0:64]))
                act(nc.scalar.copy(UFM0[:, :], TPX[:, :]))
                # absorber: pick up ufm0's completion so the later TPX
                # read-read serializer deps (chunks 8/9) are pre-covered
                act(nc.scalar.copy(SCRA[0:32, 8:12], UFM0[0:32, 0:4]))
                pe(nc.tensor.matmul(
                    P[:, :], UFM9[0:1, 0:64], KS2[0:1, KS2_H:KS2_H + YR],
                    start=True, stop=True, skip_group_check=True))
                for c in range(4):
                    c0, c1, ylo, yhi = MM2_CHUNKS[c]
                    pe(nc.tensor.matmul(
                        P[:, ylo:yhi], UFM0[0:128, 64 * c:64 * c + 64],
                        KS2[0:128, KS2_OFF[c]:KS2_OFF[c + 1]],
                        start=False, stop=True,
                        skip_group_check=True))
                # chunks 4..9 from U_B, grouped (2 transposes -> copy ->
                # 2 matmuls) so MM2 starts as soon as each pair lands
                def t1(c, dst, col):
                    cw = 128 if c < 9 else LAST_CW
                    pe(nc.tensor.transpose(
                        dst[0:cw, col:col + 64],
                        U_B[:, 128 * (c - 4):128 * (c - 4) + cw],
                        IDENT[0:64, 0:64]))

                def mm2(c, tile, col, stop=False):
                    rows = 128 if c < 7 else 116
                    c0, c1, ylo, yhi = MM2_CHUNKS[c]
                    pe(nc.tensor.matmul(
                        P[:, ylo:yhi], tile[0:rows, col:col + 64],
                        KS2[0:rows, KS2_OFF[c]:KS2_OFF[c + 1]],
                        start=False, stop=True,
                        skip_group_check=True))

                t1(4, TP45, 0)
                t1(5, TP45, 64)
                t1(6, TP67, 0)
                t1(7, TP67, 64)
                act(nc.scalar.copy(UFM45[:, :], TP45[:, :]))
                act(nc.scalar.copy(UFM67[:, :], TP67[:, :]))
                mm2(4, UFM45, 0)
                mm2(5, UFM45, 64)
                mm2(6, UFM67, 0)
                mm2(7, UFM67, 64)

                # ---- y update: y = relu(YP + 2P);  YP' = y - P ----
                dve(nc.vector.scalar_tensor_tensor(
                    T3[:, :], P[:, :], 2.0, YP[:, :], op.mult, op.add))
                dve(nc.vector.scalar_tensor_tensor(
                    T3[:, 1::2], U_B[:, 500:FB], -2.0 * float(TAUS[0]),
                    T3[:, 1::2], op.mult, op.add))
                dve(nc.vector.tensor_scalar_max(YBM[:, :], T3[:, :], 0.0))
                # ---- transpose y to feat-major ----
                for ci in range(3):
                    pe(nc.tensor.transpose(
                        TY[0:96, 64 * ci:64 * ci + 64],
                        YBM[:, 96 * ci:96 * ci + 96],
                        IDENT[0:64, 0:64]))
                for ci in range(3):
                    act(nc.scalar.copy(
                        YFMb[ci][0:96, 0:64],
                        TY[0:96, 64 * ci:64 * ci + 64]))
                # off-critical: YP for next iter (reads P psum, so DVE)
                dve(nc.vector.scalar_tensor_tensor(
                    YP[:, :], P[:, :], -1.0, YBM[:, :], op.mult, op.add))
                dve(nc.vector.scalar_tensor_tensor(
                    YP[:, 1::2], U_B[:, 500:FB], float(TAUS[0]),
                    YP[:, 1::2], op.mult, op.add))

            zdma = nc.sync.dma_start(d_z[:, :], U_B[:, FB - N:FB])
            # tail fence: the framework drain waits on every proc, but the
            # ISA allows one wait per instruction — absorb them one at a
            # time with SP nops so the drain's own waits are elided.
            for d in (dma1, dma2, prev["pool"], prev["act"], prev["pe"],
                      prev["dve"], zdma):
                nn = nc.sync.nop()
                add_dep_helper(nn.ins, d.ins, True, "tail fence")
    return nc


LAST_RESULT = None


def kernel(weights, A, A_pos, b, _trace=False):
    weights = np.asarray(weights, np.float32)
    A = np.asarray(A, np.float32)
    A_pos = np.asarray(A_pos, np.float32)
    b = np.asarray(b, np.float32)

    K, h, tau = _build_constants(A, A_pos, b)
    TAUS[0] = tau
    ka1, ks2 = _host_tiles(K, h, tau)
    yp0 = np.broadcast_to((tau * h)[Y_PERM], (64, YR)).astype(
        np.float32).copy()

    nc = _build_bass()

    in_maps = []
    for core in range(N_CORES):
        w_shard = weights[core * BS:(core + 1) * BS].reshape(BS, N)
        tq = _per_core_tiles(w_shard, tau)
        cr, cf = _pack_consts(ka1, ks2, tq, yp0)
        in_maps.append({"constsr": cr, "constsf": cf})

    from concourse.bass_utils import run_bass_kernel_spmd
    res = run_bass_kernel_spmd(nc, in_maps, core_ids=list(range(N_CORES)),
                               trace=_trace)
    global LAST_RESULT
    LAST_RESULT = res
    outs = [np.asarray(res.results[c]["z"]) for c in range(N_CORES)]
    z = np.concatenate(outs, axis=0).reshape(B, N_GRID, N_GRID)
    return z.astype(np.float32)


if __name__ == "__main__":
    TAUS[0] = 0.1
    rng = np.random.default_rng(0)
    w = rng.random((B, N_GRID, N_GRID), np.float32)
    # smoke build only
    _build_bass()
    print("bass build OK")

